# revision 1
# baseline (speedup 1.0000x reference)
"""Trainium2 Bass kernel for the LogicMessagePassingNetwork problem.

Reference computation (E=1M edges, T=2M triangles, R=50, D=64):
    x   = edge_feat + relation_emb[edge_rel]                      # [E, D]
    m   = relu((x[edge_ab] * x[edge_bc]) @ W_msg)                 # [T, D]
    agg = segment_sum(m, edge_ac, E)                              # [E, D]
    out = relu(x + agg @ W_upd)                                   # [E, D]

Strategy (8 cores):
  - Host sorts triangles by edge_ac; core k owns output edges
    [k*E/8, (k+1)*E/8) and exactly the triangles whose ac falls there.
    No cross-core communication needed (sharding by aggregation target).
  - Per core, output edges are processed in blocks of 128. Each block's
    triangles (avg 256) are padded to TB chunks of 128 triangle slots.
  - Per chunk: x rows for ab/bc sides are fetched with two indirect DMAs
    from a combined table [feat | rel_emb | zero-row] (2nd DMA does a
    CCE add of the relation row), prod = x_ab*x_bc on DVE, transposed on
    PE, m = relu(prodT^T @ W_msg) on PE+ACT, then a one-hot scatter
    matmul accumulates aggT[64, 128] in PSUM across the block's chunks.
  - Per block: out_blk = relu(x_own + (aggT^T @ W_upd)) fused, written
    straight to DRAM. Edges with no triangles work naturally (PSUM
    cleared by start=True on the first accumulating matmul).
"""
import numpy as np

E = 1_000_000
T = 2_000_000
R = 50
D = 64
NCORES = 8
EPC = E // NCORES          # edges per core
BLK = 128                  # output edges per block
NBLK = (EPC + BLK - 1) // BLK          # 977 blocks/core
EPAD = NBLK * BLK                      # padded edges/core (125056)
ZROW = E + R               # index of the all-zero row in the combined table


# ----------------------------------------------------------------- host prep
def host_preprocess(edge_rel, edge_ab, edge_bc, edge_ac, tb_override=None):
    """Pure index-space preprocessing. Returns per-core int/float index
    arrays + the chosen TB (chunks per block)."""
    edge_rel = np.asarray(edge_rel).astype(np.int64)
    ab = np.asarray(edge_ab).astype(np.int64)
    bc = np.asarray(edge_bc).astype(np.int64)
    ac = np.asarray(edge_ac).astype(np.int64)

    order = np.argsort(ac, kind="stable")
    ab_s, bc_s, ac_s = ab[order], bc[order], ac[order]

    # block id of each triangle globally: ac // BLK, blocks per core = NBLK
    blk_of_tri = ac_s // BLK
    counts = np.bincount(blk_of_tri, minlength=NCORES * NBLK)
    # NOTE: blocks are global 0..(NCORES*NBLK-1) only if EPC % BLK == 0.
    # EPC=125000, BLK=128 -> not divisible; block boundaries differ per core.
    # Redo per core below instead.

    per_core = []
    max_cnt = 0
    for k in range(NCORES):
        lo, hi = np.searchsorted(ac_s, [k * EPC, (k + 1) * EPC])
        c_ab, c_bc, c_ac = ab_s[lo:hi], bc_s[lo:hi], ac_s[lo:hi] - k * EPC
        cblk = c_ac // BLK
        ccnt = np.bincount(cblk, minlength=NBLK)
        max_cnt = max(max_cnt, int(ccnt.max()) if len(ccnt) else 0)
        per_core.append((c_ab, c_bc, c_ac, ccnt))

    TB = tb_override or -(-max_cnt // 128)      # chunks per block
    NT = NBLK * TB * 128                        # padded triangle slots/core

    outs = []
    for k in range(NCORES):
        c_ab, c_bc, c_ac, ccnt = per_core[k]
        # slot position for each triangle: block*TB*128 + rank within block
        starts = np.zeros(NBLK, np.int64)
        starts[1:] = np.cumsum(ccnt)[:-1]
        rank = np.arange(len(c_ac)) - starts[c_ac // BLK]
        slot = (c_ac // BLK) * (TB * 128) + rank

        gab = np.full(NT, ZROW, np.int32)
        gbc = np.full(NT, ZROW, np.int32)
        grel_ab = np.full(NT, ZROW, np.int32)
        grel_bc = np.full(NT, ZROW, np.int32)
        acrel = np.full(NT, 999.0, np.float32)
        gab[slot] = c_ab
        gbc[slot] = c_bc
        grel_ab[slot] = E + edge_rel[c_ab]
        grel_bc[slot] = E + edge_rel[c_bc]
        acrel[slot] = (c_ac % BLK).astype(np.float32)

        # reshape to chunk layout: [NBLK*TB, 128] -> partition-major [.., 128, 1]
        idx_ab = np.stack([gab, grel_ab], axis=-1).reshape(NBLK * TB, 128, 2)
        idx_bc = np.stack([gbc, grel_bc], axis=-1).reshape(NBLK * TB, 128, 2)
        acrel = acrel.reshape(NBLK * TB, 128, 1)

        # own-edge x: feat rows + rel rows for edges [k*EPC, k*EPC+EPAD)
        own_lo = k * EPC
        own_idx = np.arange(own_lo, own_lo + EPAD)
        valid = own_idx < E
        own_feat = np.where(valid, own_idx, ZROW).astype(np.int32)
        own_rel = np.where(valid, E + edge_rel[np.minimum(own_idx, E - 1)],
                           ZROW).astype(np.int32)
        own = np.stack([own_feat, own_rel], axis=-1).reshape(NBLK, 128, 2)
        outs.append(dict(idx_ab=idx_ab, idx_bc=idx_bc, acrel=acrel, own=own))
    return outs, TB


def build_table(edge_feat, relation_emb):
    tbl = np.zeros((E + R + 1, D), np.float32)
    tbl[:E] = edge_feat
    tbl[E:E + R] = relation_emb
    return tbl


# ------------------------------------------------------------- device kernel
def build_bass(TB, nblk, dt_str="float32"):
    """Build the SPMD Bass program. nblk = number of blocks to emit
    (use < NBLK for scaled-down testing)."""
    import concourse.bass as bass
    import concourse.bacc as bacc
    import concourse.mybir as mybir
    import concourse.tile as tile
    from concourse.masks import make_identity

    dt = getattr(mybir.dt, dt_str)
    f32 = mybir.dt.float32
    nc = bacc.Bacc(None, target_bir_lowering=False)

    tbl = nc.dram_tensor("tbl", [E + R + 1, D], f32, kind="ExternalInput")
    wmsg = nc.dram_tensor("wmsg", [D, D], f32, kind="ExternalInput")
    wupd = nc.dram_tensor("wupd", [D, D], f32, kind="ExternalInput")
    iota = nc.dram_tensor("iota", [128, 128], f32, kind="ExternalInput")
    idx_ab = nc.dram_tensor("idx_ab", [nblk * TB, 128, 2], mybir.dt.int32, kind="ExternalInput")
    idx_bc = nc.dram_tensor("idx_bc", [nblk * TB, 128, 2], mybir.dt.int32, kind="ExternalInput")
    acrel = nc.dram_tensor("acrel", [nblk * TB, 128, 1], f32, kind="ExternalInput")
    own = nc.dram_tensor("own", [nblk, 128, 2], mybir.dt.int32, kind="ExternalInput")
    out = nc.dram_tensor("out", [nblk, 128, D], f32, kind="ExternalOutput")

    with tile.TileContext(nc) as tc:
        with tc.tile_pool(name="const", bufs=1) as cpool, \
             tc.tile_pool(name="gath", bufs=16) as gpool, \
             tc.tile_pool(name="idxp", bufs=16) as ipool, \
             tc.tile_pool(name="work", bufs=8) as wpool, \
             tc.tile_pool(name="outp", bufs=6) as opool, \
             tc.tile_pool(name="ps", bufs=2, space="PSUM") as pspool, \
             tc.tile_pool(name="psagg", bufs=2, space="PSUM") as paggpool:

            wmsg_sb = cpool.tile([D, D], f32)
            nc.sync.dma_start(out=wmsg_sb[:], in_=wmsg[:])
            wupd_sb = cpool.tile([D, D], f32)
            nc.sync.dma_start(out=wupd_sb[:], in_=wupd[:])
            iota_sb = cpool.tile([128, 128], f32)
            nc.sync.dma_start(out=iota_sb[:], in_=iota[:])
            ident = cpool.tile([128, 128], f32)
            make_identity(nc, ident[:])

            for b in range(nblk):
                aggT = paggpool.tile([D, 128], f32, space="PSUM", tag="aggT")
                for c in range(TB):
                    ch = b * TB + c
                    ia = ipool.tile([128, 2], mybir.dt.int32, tag="ia")
                    nc.sync.dma_start(out=ia[:], in_=idx_ab[ch])
                    ib = ipool.tile([128, 2], mybir.dt.int32, tag="ib")
                    nc.sync.dma_start(out=ib[:], in_=idx_bc[ch])
                    ar = ipool.tile([128, 1], f32, tag="ar")
                    nc.sync.dma_start(out=ar[:], in_=acrel[ch])

                    xa = gpool.tile([128, D], f32, tag="xa")
                    nc.gpsimd.indirect_dma_start(
                        out=xa[:], out_offset=None, in_=tbl[:],
                        in_offset=bass.IndirectOffsetOnAxis(ap=ia[:, 0:1], axis=0))
                    nc.gpsimd.indirect_dma_start(
                        out=xa[:], out_offset=None, in_=tbl[:],
                        in_offset=bass.IndirectOffsetOnAxis(ap=ia[:, 1:2], axis=0),
                        compute_op=mybir.AluOpType.add)
                    xb = gpool.tile([128, D], f32, tag="xb")
                    nc.gpsimd.indirect_dma_start(
                        out=xb[:], out_offset=None, in_=tbl[:],
                        in_offset=bass.IndirectOffsetOnAxis(ap=ib[:, 0:1], axis=0))
                    nc.gpsimd.indirect_dma_start(
                        out=xb[:], out_offset=None, in_=tbl[:],
                        in_offset=bass.IndirectOffsetOnAxis(ap=ib[:, 1:2], axis=0),
                        compute_op=mybir.AluOpType.add)
                    prod = wpool.tile([128, D], f32, tag="prod")
                    nc.vector.tensor_mul(out=prod[:], in0=xa[:], in1=xb[:])

                    # prodT [64, 128] via PE transpose
                    prodT_ps = pspool.tile([D, 128], f32, space="PSUM", tag="prodT")
                    nc.tensor.transpose(out=prodT_ps[:], in_=prod[:], identity=ident[:])
                    prodT = wpool.tile([D, 128], f32, tag="prodTs")
                    nc.scalar.activation(out=prodT[:], in_=prodT_ps[:], func=mybir.ActivationFunctionType.Copy)

                    # m = relu(prod @ W_msg) : lhsT=prodT [64d,128t], rhs=wmsg
                    m_ps = pspool.tile([128, D], f32, space="PSUM", tag="mps")
                    nc.tensor.matmul(out=m_ps[:], lhsT=prodT[:], rhs=wmsg_sb[:],
                                     start=True, stop=True)
                    m_sb = wpool.tile([128, D], f32, tag="msb")
                    nc.scalar.activation(out=m_sb[:], in_=m_ps[:],
                                         func=mybir.ActivationFunctionType.Relu)

                    # one-hot [128t, 128e]
                    oh = wpool.tile([128, 128], f32, tag="oh")
                    nc.vector.tensor_tensor(out=oh[:], in0=ar[:].to_broadcast([128, 128]),
                                            in1=iota_sb[:], op=mybir.AluOpType.is_equal)
                    # aggT += m^T @ onehot
                    nc.tensor.matmul(out=aggT[:], lhsT=m_sb[:], rhs=oh[:],
                                     start=(c == 0), stop=(c == TB - 1))

                # ---- block epilogue ----
                aggT_sb = wpool.tile([D, 128], f32, tag="aggTs")
                nc.vector.tensor_copy(out=aggT_sb[:], in_=aggT[:])
                upd_ps = pspool.tile([128, D], f32, space="PSUM", tag="upd")
                nc.tensor.matmul(out=upd_ps[:], lhsT=aggT_sb[:], rhs=wupd_sb[:],
                                 start=True, stop=True)

                io = ipool.tile([128, 2], mybir.dt.int32, tag="io")
                nc.sync.dma_start(out=io[:], in_=own[b])
                xo = gpool.tile([128, D], f32, tag="xo")
                nc.gpsimd.indirect_dma_start(
                    out=xo[:], out_offset=None, in_=tbl[:],
                    in_offset=bass.IndirectOffsetOnAxis(ap=io[:, 0:1], axis=0))
                nc.gpsimd.indirect_dma_start(
                    out=xo[:], out_offset=None, in_=tbl[:],
                    in_offset=bass.IndirectOffsetOnAxis(ap=io[:, 1:2], axis=0),
                    compute_op=mybir.AluOpType.add)

                ob2 = opool.tile([128, D], f32, tag="ob2")
                nc.vector.tensor_add(out=ob2[:], in0=xo[:], in1=upd_ps[:])
                ob3 = opool.tile([128, D], f32, tag="ob3")
                nc.scalar.activation(out=ob3[:], in_=ob2[:],
                                     func=mybir.ActivationFunctionType.Relu)
                nc.sync.dma_start(out=out[b], in_=ob3[:])

    nc.compile()
    return nc


def run_full(inputs, nblk=NBLK, tb_override=None, repeats=0):
    """Full kernel entry: takes reference inputs, returns [E, D] output."""
    from concourse.bass_utils import run_bass_kernel_spmd
    pre, TB = host_preprocess(inputs["edge_rel"], inputs["edge_ab"],
                              inputs["edge_bc"], inputs["edge_ac"],
                              tb_override=tb_override)
    tbl = build_table(np.asarray(inputs["edge_feat"], np.float32),
                      np.asarray(inputs["relation_emb"], np.float32))
    iota = np.tile(np.arange(128, dtype=np.float32), (128, 1))
    import time as _time
    t0 = _time.time()
    nc = build_bass(TB, nblk)
    print(f"[build+compile {_time.time()-t0:.1f}s TB={TB}]", flush=True)
    in_maps = []
    for k in range(NCORES):
        p = pre[k]
        in_maps.append({
            "tbl": tbl,
            "wmsg": np.asarray(inputs["W_msg"], np.float32),
            "wupd": np.asarray(inputs["W_upd"], np.float32),
            "iota": iota,
            "idx_ab": p["idx_ab"][:nblk * TB],
            "idx_bc": p["idx_bc"][:nblk * TB],
            "acrel": p["acrel"][:nblk * TB],
            "own": p["own"][:nblk],
        })
    import time as _time
    t0 = _time.time()
    res = run_bass_kernel_spmd(nc, in_maps, core_ids=list(range(NCORES)))
    print(f"[run1 {_time.time()-t0:.1f}s]", flush=True)
    if repeats:
        for r in range(repeats):
            t0 = _time.time()
            res = run_bass_kernel_spmd(nc, in_maps, core_ids=list(range(NCORES)))
            print(f"[run{r+2} {_time.time()-t0:.1f}s]", flush=True)
    outs = [res.results[k]["out"].reshape(-1, D) for k in range(NCORES)]
    full = np.concatenate([o[:EPC] for o in outs], axis=0)
    return full


# ------------------------------------------------------------------ entry
def kernel(**inputs):
    """Self-contained entry: full unsharded inputs -> full [E, D] output.

    Shards internally across 8 NeuronCores (triangles sharded by their
    aggregation target edge, params/tables replicated), runs the Bass
    kernel via run_bass_kernel_spmd, and reassembles the full output.
    """
    out = run_full(inputs, nblk=NBLK)
    return out.astype(np.float32)

